# revision 2
# baseline (speedup 1.0000x reference)
"""GCN message passing (SpMM) on 8 Trainium2 NeuronCores.

out[r, :] = sum_{e: rows[e]==r} vals[e] * x[cols[e], :]

Sharding: 1D row partitioning. adj_rows is sorted, so core k owns output rows
[k*12500, (k+1)*12500) and the contiguous edge range hitting those rows.
No collectives; each core writes its own output slab.

v4a (vs v3 baseline):
  - The 4 buckets' gathers go to 4 separate SWDGE queues (one queue
    serializes all gathers end-to-end: measured 41 GB/s vs >400 GB/s).
  - dma_scatter_add replaced by a plain HWDGE dma_start per chunk: the PSUM
    result is written to DRAM in PSUM-slot order ([n_chunks*960, 48] per
    core) and the host applies the slot->row permutation while unsharding.
    Removes the SWDGE scatter (RMW traffic + queue serialization), the
    4-slab summing, the zeros init and the dump row.

Per-core algorithm (windowed 4-bucket, metadata fully SBUF-resident):
  - x is padded to [100000, 64] f32 (256B rows) and split into 4 node-range
    buckets of 25000 rows so dma_gather's int16 indices can address each.
  - Host greedily groups consecutive output rows into "windows" (<=32 rows,
    <=128 edges per bucket per window). Each (window, bucket) is one
    128-edge gather tile (tail-padded with zero-val edges).
  - All per-edge metadata (gather indices, vals, slot ids) is preloaded into
    SBUF once at kernel start, so the steady-state loop issues only:
    4 dma_gathers + 8 DVE ops + 120 matmuls + 1 ACT copy + 1 dma_start per
    30-window chunk.
  - PE accumulates the 4 buckets' S^T @ G into one PSUM [96,480] bank per
    chunk (3 window groups x 10) => full segment sums.
"""

import numpy as np

import concourse.bass as bass
import concourse.bacc as bacc
import concourse.mybir as mybir
import concourse.tile as tile
from concourse.bass_utils import run_bass_kernel_spmd

# ---------------- problem constants (hardcoded per the task contract) -------
N_NODES = 100000
D = 48
N_CORES = 8
R_PER_CORE = N_NODES // N_CORES  # 12500

# ---------------- kernel hyperparameters -----------------------------------
NB = 4               # node-range buckets (int16 gather indices: 25000 < 32768)
B_NODES = N_NODES // NB
EDGE_CAP = 128       # edges per (window, bucket) tile = PE contraction dim
SEG_CAP = 32         # max rows per window (= matmul M, psum partition group)
GP = 3               # usable 32-partition psum groups (offset 96 unusable)
CW = 30              # windows per chunk (= one PSUM bank: 3 groups x 10)
SC_H = CW // GP      # free blocks per bank (10)
EL = 64              # padded x row, f32 elements (256B)
SLOTS = 96 * SC_H    # output slots per chunk (960)

_F32 = mybir.dt.float32
_I16 = mybir.dt.int16

_NIG = CW * EDGE_CAP          # gather indices per (chunk, bucket) = 3840
_GI_W = _NIG // 16            # 240 int16 per partition per chunk


def _wrap16(flat, reps=8):
    """[(n)] int16 -> [16*reps, n/16] in the 16-partition wrap, replicated."""
    n = flat.shape[0]
    w = flat.reshape(n // 16, 16).T  # [16, n/16]
    return np.tile(w, (reps, 1))


# ===========================================================================
# Host-side prep: pure index/layout transformation (no float math on data).
# ===========================================================================
def _pack_core(rows_l, cols, vals, r_per_core):
    bucket = (cols // B_NODES).astype(np.int64)
    col_loc = (cols - bucket * B_NODES).astype(np.int16)

    cnt = np.zeros((r_per_core, NB), np.int64)
    np.add.at(cnt, (rows_l, bucket), 1)
    assert cnt.max() <= EDGE_CAP, "row degree exceeds tile capacity"

    # greedy windows over consecutive rows: <=SEG_CAP rows, <=EDGE_CAP
    # edges per bucket per window
    window_of_row = np.empty(r_per_core, np.int64)
    slot_of_row = np.empty(r_per_core, np.int64)
    w = 0
    acc = np.zeros(NB, np.int64)
    nrows = 0
    for r in range(r_per_core):
        c = cnt[r]
        if nrows == SEG_CAP or (acc + c > EDGE_CAP).any():
            w += 1
            acc[:] = 0
            nrows = 0
        window_of_row[r] = w
        slot_of_row[r] = nrows
        acc += c
        nrows += 1
    n_win = w + 1

    w_e = window_of_row[rows_l]
    slot_e = slot_of_row[rows_l].astype(np.float32)

    per_bucket = []
    for b in range(NB):
        sel = np.flatnonzero(bucket == b)
        o = np.argsort(w_e[sel], kind="stable")
        sel = sel[o]
        wb = w_e[sel]                       # non-decreasing after sort
        first = np.searchsorted(wb, np.arange(n_win))
        pos = np.arange(sel.shape[0]) - first[wb]
        assert pos.max(initial=0) < EDGE_CAP
        colb = np.zeros((n_win, EDGE_CAP), np.int16)
        valb = np.zeros((n_win, EDGE_CAP), np.float32)
        slotb = np.zeros((n_win, EDGE_CAP), np.float32)
        colb[wb, pos] = col_loc[sel]
        valb[wb, pos] = vals[sel]
        slotb[wb, pos] = slot_e[sel]
        per_bucket.append((colb, valb, slotb))

    # slot-order output: row r lives at chunk-slot p*SC_H + j where
    # w = window_of_row[r] = c*CW + 3*j + a, p = 32*a + slot_of_row[r]
    c_of = window_of_row // CW
    wl = window_of_row - c_of * CW
    a = wl % GP
    j = wl // GP
    p = 32 * a + slot_of_row
    slot_global = c_of * SLOTS + p * SC_H + j  # [r_per_core]
    return per_bucket, slot_global, n_win


def prep_inputs(adj_rows, adj_cols, adj_vals):
    """Shard + pack. Returns (per-core in_map list, per-core slot maps,
    n_chunks)."""
    adj_rows = np.asarray(adj_rows).astype(np.int64)
    adj_cols = np.asarray(adj_cols).astype(np.int64)
    adj_vals = np.asarray(adj_vals).astype(np.float32)

    bounds = np.searchsorted(adj_rows, np.arange(N_CORES + 1) * R_PER_CORE)
    packed = []
    for k in range(N_CORES):
        e0, e1 = bounds[k], bounds[k + 1]
        rows_l = adj_rows[e0:e1] - k * R_PER_CORE
        packed.append(_pack_core(rows_l, adj_cols[e0:e1],
                                 adj_vals[e0:e1], R_PER_CORE))

    nw_max = max(p[2] for p in packed)
    nw_pad = -(-nw_max // CW) * CW
    n_chunks = nw_pad // CW

    iota = np.broadcast_to(np.arange(SEG_CAP, dtype=np.float32),
                           (128, SEG_CAP)).copy()
    in_maps = []
    slot_maps = []
    for k in range(N_CORES):
        per_bucket, slot_global, n_win = packed[k]
        m = {"iota": iota}
        for b in range(NB):
            colb, valb, slotb = per_bucket[b]
            cb = np.zeros((nw_pad, EDGE_CAP), np.int16)
            vb = np.zeros((nw_pad, EDGE_CAP), np.float32)
            sb = np.zeros((nw_pad, EDGE_CAP), np.float32)
            cb[:n_win] = colb
            vb[:n_win] = valb
            sb[:n_win] = slotb
            # SBUF-resident layouts (one DMA each):
            # gidx: [128, n_chunks*_GI_W] int16 (16-wrap per chunk, x8)
            m[f"gidx{b}"] = np.concatenate([
                _wrap16(cb[c * CW:(c + 1) * CW].reshape(-1))
                for c in range(n_chunks)], axis=1)
            # vals/slot: [128, n_chunks*CW]; [p, c*CW+t] = edge t*128+p
            m[f"gval{b}"] = np.ascontiguousarray(
                vb.reshape(n_chunks, CW, EDGE_CAP).transpose(2, 0, 1)
                .reshape(128, n_chunks * CW))
            m[f"gslot{b}"] = np.ascontiguousarray(
                sb.reshape(n_chunks, CW, EDGE_CAP).transpose(2, 0, 1)
                .reshape(128, n_chunks * CW))
        in_maps.append(m)
        slot_maps.append(slot_global)
    return in_maps, slot_maps, n_chunks


def pad_x(x):
    x64 = np.zeros((N_NODES, EL), np.float32)
    x64[:, :D] = x
    return x64


# ===========================================================================
# Device program (shared across all 8 cores)
# ===========================================================================
def build_program(n_chunks):
    nc = bacc.Bacc("TRN2", target_bir_lowering=False, debug=False,
                   num_devices=N_CORES, num_swdge_queues=4)
    x_d = nc.dram_tensor("x64", [N_NODES, EL], _F32, kind="ExternalInput")
    gidx_d = [nc.dram_tensor(f"gidx{b}", [128, n_chunks * _GI_W], _I16,
                             kind="ExternalInput") for b in range(NB)]
    gval_d = [nc.dram_tensor(f"gval{b}", [128, n_chunks * CW], _F32,
                             kind="ExternalInput") for b in range(NB)]
    gslot_d = [nc.dram_tensor(f"gslot{b}", [128, n_chunks * CW], _F32,
                              kind="ExternalInput") for b in range(NB)]
    iota_d = nc.dram_tensor("iota", [128, SEG_CAP], _F32,
                            kind="ExternalInput")
    out_d = nc.dram_tensor("out", [n_chunks * SLOTS, D], _F32,
                           kind="ExternalOutput")

    with tile.TileContext(nc) as tc:
        with (
            tc.tile_pool(name="meta", bufs=1) as meta,
            tc.tile_pool(name="gbuf", bufs=2) as gbuf,
            tc.tile_pool(name="sbuf_s", bufs=2) as sbuf_s,
            tc.tile_pool(name="obuf", bufs=2) as obuf,
            tc.tile_pool(name="psum", bufs=3, space="PSUM") as psum,
        ):
            iota_t = meta.tile([128, SEG_CAP], _F32)
            nc.sync.dma_start(out=iota_t[:], in_=iota_d[:])
            gi_all, gv_all, gs_all = [], [], []
            for b in range(NB):
                gi = meta.tile([128, n_chunks * _GI_W], _I16, tag=f"giA{b}")
                gv = meta.tile([128, n_chunks * CW], _F32, tag=f"gvA{b}")
                gs = meta.tile([128, n_chunks * CW], _F32, tag=f"gsA{b}")
                nc.sync.dma_start(out=gi[:], in_=gidx_d[b][:])
                nc.sync.dma_start(out=gv[:], in_=gval_d[b][:])
                nc.sync.dma_start(out=gs[:], in_=gslot_d[b][:])
                gi_all.append(gi)
                gv_all.append(gv)
                gs_all.append(gs)

            for c in range(n_chunks):
                g_ts, s_ts = [], []
                for b in range(NB):
                    g_t = gbuf.tile([128, CW * EL], _F32, tag=f"g{b}")
                    nc.gpsimd.dma_gather(
                        out_ap=g_t[:].rearrange("p (t f) -> p t f", f=EL),
                        in_ap=x_d[B_NODES * b:B_NODES * (b + 1)],
                        idxs_ap=gi_all[b][:, c * _GI_W:(c + 1) * _GI_W],
                        num_idxs=_NIG, num_idxs_reg=_NIG, elem_size=EL,
                        single_packet=False, queue_num=b,
                    )
                    g_ts.append(g_t)

                    s_t = sbuf_s.tile([128, CW * SEG_CAP], _F32, tag=f"s{b}")
                    s3 = s_t[:].rearrange("p (t s) -> p t s", s=SEG_CAP)
                    gs_b = gs_all[b][:, c * CW:(c + 1) * CW].unsqueeze(
                        2).to_broadcast([128, CW, SEG_CAP])
                    io_b = iota_t[:].unsqueeze(1).to_broadcast(
                        [128, CW, SEG_CAP])
                    gv_b = gv_all[b][:, c * CW:(c + 1) * CW].unsqueeze(
                        2).to_broadcast([128, CW, SEG_CAP])
                    nc.vector.tensor_tensor(out=s3, in0=gs_b, in1=io_b,
                                            op=mybir.AluOpType.is_equal)
                    nc.vector.tensor_tensor(out=s3, in0=s3, in1=gv_b,
                                            op=mybir.AluOpType.mult)
                    s_ts.append(s_t)

                ps = psum.tile([128, SC_H * D], _F32, space="PSUM", tag="ps")
                for wl in range(CW):
                    a, j = wl % GP, wl // GP
                    for b in range(NB):
                        nc.tensor.matmul(
                            out=ps[32 * a:32 * a + SEG_CAP, D * j:D * j + D],
                            lhsT=s_ts[b][:, SEG_CAP * wl:SEG_CAP * (wl + 1)],
                            rhs=g_ts[b][:, EL * wl:EL * wl + D],
                            start=(b == 0), stop=(b == NB - 1),
                            skip_group_check=True,
                        )

                # PSUM [96, 10, 48] -> SBUF, then one contiguous-ish HWDGE
                # write: chunk-slot p*SC_H+j -> DRAM row c*SLOTS + p*SC_H + j
                o_t = obuf.tile([128, SC_H * D], _F32, tag="o")
                o3 = o_t[:].rearrange("p (j f) -> p j f", f=D)
                ps3 = ps[:].rearrange("p (j f) -> p j f", f=D)
                nc.scalar.copy(out=o3[:96], in_=ps3[:96])
                nc.sync.dma_start(
                    out=out_d[c * SLOTS:(c + 1) * SLOTS].rearrange(
                        "(p j) f -> p j f", j=SC_H),
                    in_=o3[:96],
                )
    nc.compile()
    return nc


# ===========================================================================
# Entry point
# ===========================================================================
_CACHE = {}


def _get_program(n_chunks):
    if n_chunks not in _CACHE:
        _CACHE[n_chunks] = build_program(n_chunks)
    return _CACHE[n_chunks]


def _run(adj_rows, adj_cols, adj_vals, x):
    x64 = pad_x(np.ascontiguousarray(np.asarray(x), dtype=np.float32))
    in_maps, slot_maps, n_chunks = prep_inputs(adj_rows, adj_cols, adj_vals)
    for m in in_maps:
        m["x64"] = x64
    nc = _get_program(n_chunks)
    res = run_bass_kernel_spmd(nc, in_maps, core_ids=list(range(N_CORES)))
    out = np.empty((N_NODES, D), np.float32)
    for k in range(N_CORES):
        out[k * R_PER_CORE:(k + 1) * R_PER_CORE] = \
            res.results[k]["out"][slot_maps[k]]
    return out, res, (in_maps, n_chunks)


def kernel(adj_rows, adj_cols, adj_vals, x):
    out, _, _ = _run(adj_rows, adj_cols, adj_vals, x)
    return out


# revision 4
# speedup vs baseline: 1.6170x; 1.6170x over previous
"""GCN message passing (SpMM) on 8 Trainium2 NeuronCores.

out[r, :] = sum_{e: rows[e]==r} vals[e] * x[cols[e], :]

Sharding: 1D row partitioning. adj_rows is sorted, so core k owns output rows
[k*12500, (k+1)*12500) and the contiguous edge range hitting those rows.
No collectives; each core writes its own output slab.

v4a (vs v3 baseline):
  - The 4 buckets' gathers go to 4 separate SWDGE queues (one queue
    serializes all gathers end-to-end: measured 41 GB/s vs >400 GB/s).
  - dma_scatter_add replaced by a plain HWDGE dma_start per chunk: the PSUM
    result is written to DRAM in PSUM-slot order ([n_chunks*960, 48] per
    core) and the host applies the slot->row permutation while unsharding.
    Removes the SWDGE scatter (RMW traffic + queue serialization), the
    4-slab summing, the zeros init and the dump row.

Per-core algorithm (windowed 4-bucket, metadata fully SBUF-resident):
  - x is padded to [100000, 64] f32 (256B rows) and split into 4 node-range
    buckets of 25000 rows so dma_gather's int16 indices can address each.
  - Host greedily groups consecutive output rows into "windows" (<=32 rows,
    <=128 edges per bucket per window). Each (window, bucket) is one
    128-edge gather tile (tail-padded with zero-val edges).
  - All per-edge metadata (gather indices, vals, slot ids) is preloaded into
    SBUF once at kernel start, so the steady-state loop issues only:
    4 dma_gathers + 8 DVE ops + 120 matmuls + 1 ACT copy + 1 dma_start per
    30-window chunk.
  - PE accumulates the 4 buckets' S^T @ G into one PSUM [96,480] bank per
    chunk (3 window groups x 10) => full segment sums.
"""

import numpy as np

import concourse.bass as bass
import concourse.bacc as bacc
import concourse.mybir as mybir
import concourse.tile as tile
from concourse.bass_utils import run_bass_kernel_spmd

# ---------------- problem constants (hardcoded per the task contract) -------
N_NODES = 100000
D = 48
N_CORES = 8
R_PER_CORE = N_NODES // N_CORES  # 12500

# ---------------- kernel hyperparameters -----------------------------------
NB = 4               # node-range buckets (int16 gather indices: 25000 < 32768)
B_NODES = N_NODES // NB
EDGE_CAP = 128       # edges per (window, bucket) tile = PE contraction dim
SEG_CAP = 32         # max rows per window (= matmul M, psum partition group)
GP = 3               # usable 32-partition psum groups (offset 96 unusable)
CW = 30              # windows per chunk (= one PSUM bank: 3 groups x 10)
SC_H = CW // GP      # free blocks per bank (10)
EL = 64              # padded out row, f32 elements (256B)
XE = 128             # padded x row, bf16 elements (256B)
SLOTS = 96 * SC_H    # output slots per chunk (960)

_F32 = mybir.dt.float32
_BF16 = mybir.dt.bfloat16
_I16 = mybir.dt.int16

_NIG = CW * EDGE_CAP          # gather indices per (chunk, bucket) = 3840
_GI_W = _NIG // 16            # 240 int16 per partition per chunk


def _wrap16(flat, reps=8):
    """[(n)] int16 -> [16*reps, n/16] in the 16-partition wrap, replicated."""
    n = flat.shape[0]
    w = flat.reshape(n // 16, 16).T  # [16, n/16]
    return np.tile(w, (reps, 1))


# ===========================================================================
# Host-side prep: pure index/layout transformation (no float math on data).
# ===========================================================================
def _pack_core(rows_l, cols, vals, r_per_core):
    bucket = (cols // B_NODES).astype(np.int64)
    col_loc = (cols - bucket * B_NODES).astype(np.int16)

    cnt = np.zeros((r_per_core, NB), np.int64)
    np.add.at(cnt, (rows_l, bucket), 1)
    assert cnt.max() <= EDGE_CAP, "row degree exceeds tile capacity"

    # greedy windows over consecutive rows: <=SEG_CAP rows, <=EDGE_CAP
    # edges per bucket per window
    window_of_row = np.empty(r_per_core, np.int64)
    slot_of_row = np.empty(r_per_core, np.int64)
    w = 0
    acc = np.zeros(NB, np.int64)
    nrows = 0
    for r in range(r_per_core):
        c = cnt[r]
        if nrows == SEG_CAP or (acc + c > EDGE_CAP).any():
            w += 1
            acc[:] = 0
            nrows = 0
        window_of_row[r] = w
        slot_of_row[r] = nrows
        acc += c
        nrows += 1
    n_win = w + 1

    w_e = window_of_row[rows_l]
    slot_e = slot_of_row[rows_l].astype(np.float32)

    per_bucket = []
    for b in range(NB):
        sel = np.flatnonzero(bucket == b)
        o = np.argsort(w_e[sel], kind="stable")
        sel = sel[o]
        wb = w_e[sel]                       # non-decreasing after sort
        first = np.searchsorted(wb, np.arange(n_win))
        pos = np.arange(sel.shape[0]) - first[wb]
        assert pos.max(initial=0) < EDGE_CAP
        colb = np.zeros((n_win, EDGE_CAP), np.int16)
        valb = np.zeros((n_win, EDGE_CAP), np.float32)
        slotb = np.zeros((n_win, EDGE_CAP), np.float32)
        colb[wb, pos] = col_loc[sel]
        valb[wb, pos] = vals[sel]
        slotb[wb, pos] = slot_e[sel]
        per_bucket.append((colb, valb, slotb))

    # slot-order output: row r lives at chunk-slot p*SC_H + j where
    # w = window_of_row[r] = c*CW + 3*j + a, p = 32*a + slot_of_row[r]
    c_of = window_of_row // CW
    wl = window_of_row - c_of * CW
    a = wl % GP
    j = wl // GP
    p = 32 * a + slot_of_row
    slot_global = c_of * SLOTS + p * SC_H + j  # [r_per_core]
    return per_bucket, slot_global, n_win


def prep_inputs(adj_rows, adj_cols, adj_vals):
    """Shard + pack. Returns (per-core in_map list, per-core slot maps,
    n_chunks)."""
    adj_rows = np.asarray(adj_rows).astype(np.int64)
    adj_cols = np.asarray(adj_cols).astype(np.int64)
    adj_vals = np.asarray(adj_vals).astype(np.float32)

    bounds = np.searchsorted(adj_rows, np.arange(N_CORES + 1) * R_PER_CORE)
    packed = []
    for k in range(N_CORES):
        e0, e1 = bounds[k], bounds[k + 1]
        rows_l = adj_rows[e0:e1] - k * R_PER_CORE
        packed.append(_pack_core(rows_l, adj_cols[e0:e1],
                                 adj_vals[e0:e1], R_PER_CORE))

    nw_max = max(p[2] for p in packed)
    nw_pad = -(-nw_max // CW) * CW
    n_chunks = nw_pad // CW

    import ml_dtypes
    bf16 = ml_dtypes.bfloat16
    iota = np.broadcast_to(np.arange(SEG_CAP, dtype=np.float32),
                           (128, SEG_CAP)).astype(bf16)
    in_maps = []
    slot_maps = []
    for k in range(N_CORES):
        per_bucket, slot_global, n_win = packed[k]
        m = {"iota": iota}
        for b in range(NB):
            colb, valb, slotb = per_bucket[b]
            cb = np.zeros((nw_pad, EDGE_CAP), np.int16)
            vb = np.zeros((nw_pad, EDGE_CAP), np.float32)
            sb = np.zeros((nw_pad, EDGE_CAP), np.float32)
            cb[:n_win] = colb
            vb[:n_win] = valb
            sb[:n_win] = slotb
            # SBUF-resident layouts (one DMA each):
            # gidx: [128, n_chunks*_GI_W] int16 (16-wrap per chunk, x8)
            m[f"gidx{b}"] = np.concatenate([
                _wrap16(cb[c * CW:(c + 1) * CW].reshape(-1))
                for c in range(n_chunks)], axis=1)
            # vals/slot: [128, n_chunks*CW]; [p, c*CW+t] = edge t*128+p
            m[f"gval{b}"] = np.ascontiguousarray(
                vb.reshape(n_chunks, CW, EDGE_CAP).transpose(2, 0, 1)
                .reshape(128, n_chunks * CW)).astype(bf16)
            m[f"gslot{b}"] = np.ascontiguousarray(
                sb.reshape(n_chunks, CW, EDGE_CAP).transpose(2, 0, 1)
                .reshape(128, n_chunks * CW)).astype(bf16)
        in_maps.append(m)
        slot_maps.append(slot_global)
    return in_maps, slot_maps, n_chunks


def pad_x(x):
    import ml_dtypes
    x2 = np.zeros((N_NODES, XE), ml_dtypes.bfloat16)
    x2[:, :D] = x.astype(ml_dtypes.bfloat16)
    return x2


# ===========================================================================
# Device program (shared across all 8 cores)
# ===========================================================================
def build_program(n_chunks):
    nc = bacc.Bacc("TRN2", target_bir_lowering=False, debug=False,
                   num_devices=N_CORES, num_swdge_queues=4)
    x_d = nc.dram_tensor("x64", [N_NODES, XE], _BF16, kind="ExternalInput")
    gidx_d = [nc.dram_tensor(f"gidx{b}", [128, n_chunks * _GI_W], _I16,
                             kind="ExternalInput") for b in range(NB)]
    gval_d = [nc.dram_tensor(f"gval{b}", [128, n_chunks * CW], _BF16,
                             kind="ExternalInput") for b in range(NB)]
    gslot_d = [nc.dram_tensor(f"gslot{b}", [128, n_chunks * CW], _BF16,
                              kind="ExternalInput") for b in range(NB)]
    iota_d = nc.dram_tensor("iota", [128, SEG_CAP], _BF16,
                            kind="ExternalInput")
    out_d = nc.dram_tensor("out", [n_chunks * SLOTS, D], _F32,
                           kind="ExternalOutput")

    with tile.TileContext(nc) as tc:
        with (
            tc.tile_pool(name="meta", bufs=1) as meta,
            tc.tile_pool(name="gbuf", bufs=2) as gbuf,
            tc.tile_pool(name="sbuf_s", bufs=2) as sbuf_s,
            tc.tile_pool(name="obuf", bufs=2) as obuf,
            tc.tile_pool(name="psum", bufs=3, space="PSUM") as psum,
        ):
            iota_t = meta.tile([128, SEG_CAP], _BF16)
            nc.sync.dma_start(out=iota_t[:], in_=iota_d[:])
            gi_all, gv_all, gs_all = [], [], []
            for b in range(NB):
                gi = meta.tile([128, n_chunks * _GI_W], _I16, tag=f"giA{b}")
                gv = meta.tile([128, n_chunks * CW], _BF16, tag=f"gvA{b}")
                gs = meta.tile([128, n_chunks * CW], _BF16, tag=f"gsA{b}")
                nc.sync.dma_start(out=gi[:], in_=gidx_d[b][:])
                nc.sync.dma_start(out=gv[:], in_=gval_d[b][:])
                nc.sync.dma_start(out=gs[:], in_=gslot_d[b][:])
                gi_all.append(gi)
                gv_all.append(gv)
                gs_all.append(gs)

            for c in range(n_chunks):
                g_ts, s_ts = [], []
                for b in range(NB):
                    g_t = gbuf.tile([128, CW * XE], _BF16, tag=f"g{b}")
                    nc.gpsimd.dma_gather(
                        out_ap=g_t[:].rearrange("p (t f) -> p t f", f=XE),
                        in_ap=x_d[B_NODES * b:B_NODES * (b + 1)],
                        idxs_ap=gi_all[b][:, c * _GI_W:(c + 1) * _GI_W],
                        num_idxs=_NIG, num_idxs_reg=_NIG, elem_size=XE,
                        single_packet=False, queue_num=b,
                    )
                    g_ts.append(g_t)

                    s_t = sbuf_s.tile([128, CW * SEG_CAP], _BF16, tag=f"s{b}")
                    s3 = s_t[:].rearrange("p (t s) -> p t s", s=SEG_CAP)
                    gs_b = gs_all[b][:, c * CW:(c + 1) * CW].unsqueeze(
                        2).to_broadcast([128, CW, SEG_CAP])
                    io_b = iota_t[:].unsqueeze(1).to_broadcast(
                        [128, CW, SEG_CAP])
                    gv_b = gv_all[b][:, c * CW:(c + 1) * CW].unsqueeze(
                        2).to_broadcast([128, CW, SEG_CAP])
                    nc.vector.tensor_tensor(out=s3, in0=gs_b, in1=io_b,
                                            op=mybir.AluOpType.is_equal)
                    nc.vector.tensor_tensor(out=s3, in0=s3, in1=gv_b,
                                            op=mybir.AluOpType.mult)
                    s_ts.append(s_t)

                ps = psum.tile([128, SC_H * D], _F32, space="PSUM", tag="ps")
                for wl in range(CW):
                    a, j = wl % GP, wl // GP
                    for b in range(NB):
                        nc.tensor.matmul(
                            out=ps[32 * a:32 * a + SEG_CAP, D * j:D * j + D],
                            lhsT=s_ts[b][:, SEG_CAP * wl:SEG_CAP * (wl + 1)],
                            rhs=g_ts[b][:, XE * wl:XE * wl + D],
                            start=(b == 0), stop=(b == NB - 1),
                            skip_group_check=True,
                        )

                # PSUM [96, 10, 48] -> SBUF, then one contiguous-ish HWDGE
                # write: chunk-slot p*SC_H+j -> DRAM row c*SLOTS + p*SC_H + j
                o_t = obuf.tile([128, SC_H * D], _F32, tag="o")
                o3 = o_t[:].rearrange("p (j f) -> p j f", f=D)
                ps3 = ps[:].rearrange("p (j f) -> p j f", f=D)
                nc.scalar.copy(out=o3[:96], in_=ps3[:96])
                nc.sync.dma_start(
                    out=out_d[c * SLOTS:(c + 1) * SLOTS].rearrange(
                        "(p j) f -> p j f", j=SC_H),
                    in_=o3[:96],
                )
    nc.compile()
    return nc


# ===========================================================================
# Entry point
# ===========================================================================
_CACHE = {}


def _get_program(n_chunks):
    if n_chunks not in _CACHE:
        _CACHE[n_chunks] = build_program(n_chunks)
    return _CACHE[n_chunks]


def _run(adj_rows, adj_cols, adj_vals, x):
    x64 = pad_x(np.ascontiguousarray(np.asarray(x), dtype=np.float32))
    in_maps, slot_maps, n_chunks = prep_inputs(adj_rows, adj_cols, adj_vals)
    for m in in_maps:
        m["x64"] = x64
    nc = _get_program(n_chunks)
    res = run_bass_kernel_spmd(nc, in_maps, core_ids=list(range(N_CORES)))
    out = np.empty((N_NODES, D), np.float32)
    for k in range(N_CORES):
        out[k * R_PER_CORE:(k + 1) * R_PER_CORE] = \
            res.results[k]["out"][slot_maps[k]]
    return out, res, (in_maps, n_chunks)


def kernel(adj_rows, adj_cols, adj_vals, x):
    out, _, _ = _run(adj_rows, adj_cols, adj_vals, x)
    return out


# revision 5
# speedup vs baseline: 2.3870x; 1.4762x over previous
"""GCN message passing (SpMM) on 8 Trainium2 NeuronCores.

out[r, :] = sum_{e: rows[e]==r} vals[e] * x[cols[e], :]

Sharding: 1D row partitioning. adj_rows is sorted, so core k owns output rows
[k*12500, (k+1)*12500) and the contiguous edge range hitting those rows.
No collectives; each core writes its own output slab.

v4a (vs v3 baseline):
  - The 4 buckets' gathers go to 4 separate SWDGE queues (one queue
    serializes all gathers end-to-end: measured 41 GB/s vs >400 GB/s).
  - dma_scatter_add replaced by a plain HWDGE dma_start per chunk: the PSUM
    result is written to DRAM in PSUM-slot order ([n_chunks*960, 48] per
    core) and the host applies the slot->row permutation while unsharding.
    Removes the SWDGE scatter (RMW traffic + queue serialization), the
    4-slab summing, the zeros init and the dump row.

Per-core algorithm (windowed 4-bucket, metadata fully SBUF-resident):
  - x is padded to [100000, 64] f32 (256B rows) and split into 4 node-range
    buckets of 25000 rows so dma_gather's int16 indices can address each.
  - Host greedily groups consecutive output rows into "windows" (<=32 rows,
    <=128 edges per bucket per window). Each (window, bucket) is one
    128-edge gather tile (tail-padded with zero-val edges).
  - All per-edge metadata (gather indices, vals, slot ids) is preloaded into
    SBUF once at kernel start, so the steady-state loop issues only:
    4 dma_gathers + 8 DVE ops + 120 matmuls + 1 ACT copy + 1 dma_start per
    30-window chunk.
  - PE accumulates the 4 buckets' S^T @ G into one PSUM [96,480] bank per
    chunk (3 window groups x 10) => full segment sums.
"""

import numpy as np

import concourse.bass as bass
import concourse.bacc as bacc
import concourse.mybir as mybir
import concourse.tile as tile
from concourse.bass_utils import run_bass_kernel_spmd

# ---------------- problem constants (hardcoded per the task contract) -------
N_NODES = 100000
D = 48
N_CORES = 8
R_PER_CORE = N_NODES // N_CORES  # 12500

# ---------------- kernel hyperparameters -----------------------------------
NB = 4               # node-range buckets (int16 gather indices: 25000 < 32768)
B_NODES = N_NODES // NB
EDGE_CAP = 128       # edges per (window, bucket) tile = PE contraction dim
SEG_CAP = 32         # max rows per window (= matmul M, psum partition group)
GP = 3               # usable 32-partition psum groups (offset 96 unusable)
CW = 30              # windows per chunk (= one PSUM bank: 3 groups x 10)
SC_H = CW // GP      # free blocks per bank (10)
EL = 64              # padded out row, f32 elements (256B)
XE = 128             # padded x row, bf16 elements (256B)
SLOTS = 96 * SC_H    # output slots per chunk (960)

_F32 = mybir.dt.float32
_BF16 = mybir.dt.bfloat16
_I16 = mybir.dt.int16

_NIG = CW * EDGE_CAP          # gather indices per (chunk, bucket) = 3840
_GI_W = _NIG // 16            # 240 int16 per partition per chunk


def _wrap16(flat, reps=8):
    """[(n)] int16 -> [16*reps, n/16] in the 16-partition wrap, replicated."""
    n = flat.shape[0]
    w = flat.reshape(n // 16, 16).T  # [16, n/16]
    return np.tile(w, (reps, 1))


# ===========================================================================
# Host-side prep: pure index/layout transformation (no float math on data).
# ===========================================================================
def _pack_core(rows_l, cols, vals, r_per_core):
    bucket = (cols // B_NODES).astype(np.int64)
    col_loc = (cols - bucket * B_NODES).astype(np.int16)

    cnt = np.zeros((r_per_core, NB), np.int64)
    np.add.at(cnt, (rows_l, bucket), 1)
    assert cnt.max() <= EDGE_CAP, "row degree exceeds tile capacity"

    # greedy windows over consecutive rows: <=SEG_CAP rows, <=EDGE_CAP
    # edges per bucket per window
    window_of_row = np.empty(r_per_core, np.int64)
    slot_of_row = np.empty(r_per_core, np.int64)
    w = 0
    acc = np.zeros(NB, np.int64)
    nrows = 0
    for r in range(r_per_core):
        c = cnt[r]
        if nrows == SEG_CAP or (acc + c > EDGE_CAP).any():
            w += 1
            acc[:] = 0
            nrows = 0
        window_of_row[r] = w
        slot_of_row[r] = nrows
        acc += c
        nrows += 1
    n_win = w + 1

    w_e = window_of_row[rows_l]
    slot_e = slot_of_row[rows_l].astype(np.float32)

    per_bucket = []
    for b in range(NB):
        sel = np.flatnonzero(bucket == b)
        o = np.argsort(w_e[sel], kind="stable")
        sel = sel[o]
        wb = w_e[sel]                       # non-decreasing after sort
        first = np.searchsorted(wb, np.arange(n_win))
        pos = np.arange(sel.shape[0]) - first[wb]
        assert pos.max(initial=0) < EDGE_CAP
        colb = np.zeros((n_win, EDGE_CAP), np.int16)
        valb = np.zeros((n_win, EDGE_CAP), np.float32)
        slotb = np.zeros((n_win, EDGE_CAP), np.float32)
        colb[wb, pos] = col_loc[sel]
        valb[wb, pos] = vals[sel]
        slotb[wb, pos] = slot_e[sel]
        per_bucket.append((colb, valb, slotb))

    # slot-order output: row r lives at chunk-slot p*SC_H + j where
    # w = window_of_row[r] = c*CW + 3*j + a, p = 32*a + slot_of_row[r]
    c_of = window_of_row // CW
    wl = window_of_row - c_of * CW
    a = wl % GP
    j = wl // GP
    p = 32 * a + slot_of_row
    slot_global = c_of * SLOTS + p * SC_H + j  # [r_per_core]
    return per_bucket, slot_global, n_win


def prep_inputs(adj_rows, adj_cols, adj_vals):
    """Shard + pack. Returns (per-core in_map list, per-core slot maps,
    n_chunks)."""
    adj_rows = np.asarray(adj_rows).astype(np.int64)
    adj_cols = np.asarray(adj_cols).astype(np.int64)
    adj_vals = np.asarray(adj_vals).astype(np.float32)

    bounds = np.searchsorted(adj_rows, np.arange(N_CORES + 1) * R_PER_CORE)
    packed = []
    for k in range(N_CORES):
        e0, e1 = bounds[k], bounds[k + 1]
        rows_l = adj_rows[e0:e1] - k * R_PER_CORE
        packed.append(_pack_core(rows_l, adj_cols[e0:e1],
                                 adj_vals[e0:e1], R_PER_CORE))

    nw_max = max(p[2] for p in packed)
    nw_pad = -(-nw_max // CW) * CW
    n_chunks = nw_pad // CW

    import ml_dtypes
    bf16 = ml_dtypes.bfloat16
    iota = np.broadcast_to(np.arange(SEG_CAP, dtype=np.float32),
                           (128, SEG_CAP)).astype(bf16)
    in_maps = []
    slot_maps = []
    for k in range(N_CORES):
        per_bucket, slot_global, n_win = packed[k]
        m = {"iota": iota}
        for b in range(NB):
            colb, valb, slotb = per_bucket[b]
            cb = np.zeros((nw_pad, EDGE_CAP), np.int16)
            vb = np.zeros((nw_pad, EDGE_CAP), np.float32)
            sb = np.zeros((nw_pad, EDGE_CAP), np.float32)
            cb[:n_win] = colb
            vb[:n_win] = valb
            sb[:n_win] = slotb
            # SBUF-resident layouts (one DMA each):
            # gidx: [128, n_chunks*_GI_W] int16 (16-wrap per chunk, x8)
            m[f"gidx{b}"] = np.concatenate([
                _wrap16(cb[c * CW:(c + 1) * CW].reshape(-1))
                for c in range(n_chunks)], axis=1)
            # vals/slot: [128, n_chunks*CW]; [p, c*CW+t] = edge t*128+p
            m[f"gval{b}"] = np.ascontiguousarray(
                vb.reshape(n_chunks, CW, EDGE_CAP).transpose(2, 0, 1)
                .reshape(128, n_chunks * CW)).astype(bf16)
            m[f"gslot{b}"] = np.ascontiguousarray(
                sb.reshape(n_chunks, CW, EDGE_CAP).transpose(2, 0, 1)
                .reshape(128, n_chunks * CW)).astype(bf16)
        in_maps.append(m)
        slot_maps.append(slot_global)
    return in_maps, slot_maps, n_chunks


def pad_x(x):
    import ml_dtypes
    x2 = np.zeros((N_NODES, XE), ml_dtypes.bfloat16)
    x2[:, :D] = x.astype(ml_dtypes.bfloat16)
    return x2


# ===========================================================================
# Device program (shared across all 8 cores)
# ===========================================================================
def build_program(n_chunks):
    nc = bacc.Bacc("TRN2", target_bir_lowering=False, debug=False,
                   num_devices=N_CORES, num_swdge_queues=4)
    x_d = nc.dram_tensor("x64", [N_NODES, XE], _BF16, kind="ExternalInput")
    gidx_d = [nc.dram_tensor(f"gidx{b}", [128, n_chunks * _GI_W], _I16,
                             kind="ExternalInput") for b in range(NB)]
    gval_d = [nc.dram_tensor(f"gval{b}", [128, n_chunks * CW], _BF16,
                             kind="ExternalInput") for b in range(NB)]
    gslot_d = [nc.dram_tensor(f"gslot{b}", [128, n_chunks * CW], _BF16,
                              kind="ExternalInput") for b in range(NB)]
    iota_d = nc.dram_tensor("iota", [128, SEG_CAP], _BF16,
                            kind="ExternalInput")
    out_d = nc.dram_tensor("out", [n_chunks * SLOTS, D], _F32,
                           kind="ExternalOutput")

    with tile.TileContext(nc) as tc:
        with (
            tc.tile_pool(name="meta", bufs=1) as meta,
            tc.tile_pool(name="gbuf", bufs=3) as gbuf,
            tc.tile_pool(name="sbuf_s", bufs=2) as sbuf_s,
            tc.tile_pool(name="obuf", bufs=2) as obuf,
            tc.tile_pool(name="psum", bufs=4, space="PSUM") as psum,
        ):
            iota_t = meta.tile([128, SEG_CAP], _BF16)
            nc.sync.dma_start(out=iota_t[:], in_=iota_d[:])
            gi_all, gv_all, gs_all = [], [], []
            for b in range(NB):
                gv = meta.tile([128, n_chunks * CW], _BF16, tag=f"gvA{b}")
                gs = meta.tile([128, n_chunks * CW], _BF16, tag=f"gsA{b}")
                nc.sync.dma_start(out=gv[:], in_=gval_d[b][:])
                nc.sync.dma_start(out=gs[:], in_=gslot_d[b][:])
                gv_all.append(gv)
                gs_all.append(gs)
            # chunk-major gidx slices: chunk c's gathers only wait on their
            # own [128, 240] slice, so the pipeline starts ~15us earlier
            for c in range(n_chunks):
                for b in range(NB):
                    gic = meta.tile([128, _GI_W], _I16, tag=f"gi{b}_{c}")
                    nc.sync.dma_start(
                        out=gic[:],
                        in_=gidx_d[b][:, c * _GI_W:(c + 1) * _GI_W])
                    gi_all.append(gic)

            for c in range(n_chunks):
                g_ts, s_ts = [], []
                for b in range(NB):
                    g_t = gbuf.tile([128, CW * XE], _BF16, tag=f"g{b}")
                    nc.gpsimd.dma_gather(
                        out_ap=g_t[:].rearrange("p (t f) -> p t f", f=XE),
                        in_ap=x_d[B_NODES * b:B_NODES * (b + 1)],
                        idxs_ap=gi_all[c * NB + b][:],
                        num_idxs=_NIG, num_idxs_reg=_NIG, elem_size=XE,
                        single_packet=False, queue_num=b,
                    )
                    g_ts.append(g_t)

                    s_t = sbuf_s.tile([128, CW * SEG_CAP], _BF16, tag=f"s{b}")
                    s3 = s_t[:].rearrange("p (t s) -> p t s", s=SEG_CAP)
                    gs_b = gs_all[b][:, c * CW:(c + 1) * CW].unsqueeze(
                        2).to_broadcast([128, CW, SEG_CAP])
                    io_b = iota_t[:].unsqueeze(1).to_broadcast(
                        [128, CW, SEG_CAP])
                    gv_b = gv_all[b][:, c * CW:(c + 1) * CW].unsqueeze(
                        2).to_broadcast([128, CW, SEG_CAP])
                    nc.vector.tensor_tensor(out=s3, in0=gs_b, in1=io_b,
                                            op=mybir.AluOpType.is_equal)
                    nc.vector.tensor_tensor(out=s3, in0=s3, in1=gv_b,
                                            op=mybir.AluOpType.mult)
                    s_ts.append(s_t)

                ps = psum.tile([128, SC_H * D], _F32, space="PSUM", tag="ps")
                for wl in range(CW):
                    a, j = wl % GP, wl // GP
                    for b in range(NB):
                        nc.tensor.matmul(
                            out=ps[32 * a:32 * a + SEG_CAP, D * j:D * j + D],
                            lhsT=s_ts[b][:, SEG_CAP * wl:SEG_CAP * (wl + 1)],
                            rhs=g_ts[b][:, XE * wl:XE * wl + D],
                            start=(b == 0), stop=(b == NB - 1),
                            skip_group_check=True,
                        )

                # PSUM [96, 10, 48] -> SBUF, then one contiguous-ish HWDGE
                # write: chunk-slot p*SC_H+j -> DRAM row c*SLOTS + p*SC_H + j
                o_t = obuf.tile([128, SC_H * D], _F32, tag="o")
                o3 = o_t[:].rearrange("p (j f) -> p j f", f=D)
                ps3 = ps[:].rearrange("p (j f) -> p j f", f=D)
                nc.scalar.copy(out=o3[:96], in_=ps3[:96])
                nc.sync.dma_start(
                    out=out_d[c * SLOTS:(c + 1) * SLOTS].rearrange(
                        "(p j) f -> p j f", j=SC_H),
                    in_=o3[:96],
                )
    nc.compile()
    return nc


# ===========================================================================
# Entry point
# ===========================================================================
_CACHE = {}


def _get_program(n_chunks):
    if n_chunks not in _CACHE:
        _CACHE[n_chunks] = build_program(n_chunks)
    return _CACHE[n_chunks]


def _run(adj_rows, adj_cols, adj_vals, x):
    x64 = pad_x(np.ascontiguousarray(np.asarray(x), dtype=np.float32))
    in_maps, slot_maps, n_chunks = prep_inputs(adj_rows, adj_cols, adj_vals)
    for m in in_maps:
        m["x64"] = x64
    nc = _get_program(n_chunks)
    res = run_bass_kernel_spmd(nc, in_maps, core_ids=list(range(N_CORES)))
    out = np.empty((N_NODES, D), np.float32)
    for k in range(N_CORES):
        out[k * R_PER_CORE:(k + 1) * R_PER_CORE] = \
            res.results[k]["out"][slot_maps[k]]
    return out, res, (in_maps, n_chunks)


def kernel(adj_rows, adj_cols, adj_vals, x):
    out, _, _ = _run(adj_rows, adj_cols, adj_vals, x)
    return out


# revision 6
# speedup vs baseline: 4.2248x; 1.7699x over previous
"""GCN message passing (SpMM) on 8 Trainium2 NeuronCores.

out[r, :] = sum_{e: rows[e]==r} vals[e] * x[cols[e], :]

Sharding: 1D row partitioning. adj_rows is sorted, so core k owns output rows
[k*12500, (k+1)*12500) and the contiguous edge range hitting those rows.
No collectives; each core writes its own output slab.

v4a (vs v3 baseline):
  - The 4 buckets' gathers go to 4 separate SWDGE queues (one queue
    serializes all gathers end-to-end: measured 41 GB/s vs >400 GB/s).
  - dma_scatter_add replaced by a plain HWDGE dma_start per chunk: the PSUM
    result is written to DRAM in PSUM-slot order ([n_chunks*960, 48] per
    core) and the host applies the slot->row permutation while unsharding.
    Removes the SWDGE scatter (RMW traffic + queue serialization), the
    4-slab summing, the zeros init and the dump row.

Per-core algorithm (windowed 4-bucket, metadata fully SBUF-resident):
  - x is padded to [100000, 64] f32 (256B rows) and split into 4 node-range
    buckets of 25000 rows so dma_gather's int16 indices can address each.
  - Host greedily groups consecutive output rows into "windows" (<=32 rows,
    <=128 edges per bucket per window). Each (window, bucket) is one
    128-edge gather tile (tail-padded with zero-val edges).
  - All per-edge metadata (gather indices, vals, slot ids) is preloaded into
    SBUF once at kernel start, so the steady-state loop issues only:
    4 dma_gathers + 8 DVE ops + 120 matmuls + 1 ACT copy + 1 dma_start per
    30-window chunk.
  - PE accumulates the 4 buckets' S^T @ G into one PSUM [96,480] bank per
    chunk (3 window groups x 10) => full segment sums.
"""

import numpy as np

import concourse.bass as bass
import concourse.bacc as bacc
import concourse.mybir as mybir
import concourse.tile as tile
from concourse.bass_utils import run_bass_kernel_spmd

# ---------------- problem constants (hardcoded per the task contract) -------
N_NODES = 100000
D = 48
N_CORES = 8
R_PER_CORE = N_NODES // N_CORES  # 12500

# ---------------- kernel hyperparameters -----------------------------------
NB = 4               # node-range buckets (int16 gather indices: 25000 < 32768)
B_NODES = N_NODES // NB
EDGE_CAP = 128       # edges per (window, bucket) tile = PE contraction dim
SEG_CAP = 32         # max rows per window (= matmul M, psum partition group)
GP = 3               # usable 32-partition psum groups (offset 96 unusable)
CW = 30              # windows per chunk (= one PSUM bank: 3 groups x 10)
SC_H = CW // GP      # free blocks per bank (10)
EL = 64              # padded out row, f32 elements (256B)
XE = 128             # padded x row, bf16 elements (256B)
SLOTS = 96 * SC_H    # output slots per chunk (960)

_F32 = mybir.dt.float32
_BF16 = mybir.dt.bfloat16
_I16 = mybir.dt.int16

_NIG = CW * EDGE_CAP          # gather indices per (chunk, bucket) = 3840
_GI_W = _NIG // 16            # 240 int16 per partition per chunk


def _wrap16(flat, reps=8):
    """[(n)] int16 -> [16*reps, n/16] in the 16-partition wrap, replicated."""
    n = flat.shape[0]
    w = flat.reshape(n // 16, 16).T  # [16, n/16]
    return np.tile(w, (reps, 1))


# ===========================================================================
# Host-side prep: pure index/layout transformation (no float math on data).
# ===========================================================================
def _pack_core(rows_l, cols, vals, r_per_core):
    bucket = (cols // B_NODES).astype(np.int64)
    col_loc = (cols - bucket * B_NODES).astype(np.int16)

    cnt = np.zeros((r_per_core, NB), np.int64)
    np.add.at(cnt, (rows_l, bucket), 1)
    assert cnt.max() <= EDGE_CAP, "row degree exceeds tile capacity"

    # greedy windows over consecutive rows: <=SEG_CAP rows, <=EDGE_CAP
    # edges per bucket per window
    window_of_row = np.empty(r_per_core, np.int64)
    slot_of_row = np.empty(r_per_core, np.int64)
    w = 0
    acc = np.zeros(NB, np.int64)
    nrows = 0
    for r in range(r_per_core):
        c = cnt[r]
        if nrows == SEG_CAP or (acc + c > EDGE_CAP).any():
            w += 1
            acc[:] = 0
            nrows = 0
        window_of_row[r] = w
        slot_of_row[r] = nrows
        acc += c
        nrows += 1
    n_win = w + 1

    w_e = window_of_row[rows_l]
    slot_e = slot_of_row[rows_l].astype(np.float32)

    per_bucket = []
    for b in range(NB):
        sel = np.flatnonzero(bucket == b)
        o = np.argsort(w_e[sel], kind="stable")
        sel = sel[o]
        wb = w_e[sel]                       # non-decreasing after sort
        first = np.searchsorted(wb, np.arange(n_win))
        pos = np.arange(sel.shape[0]) - first[wb]
        assert pos.max(initial=0) < EDGE_CAP
        colb = np.zeros((n_win, EDGE_CAP), np.int16)
        valb = np.zeros((n_win, EDGE_CAP), np.float32)
        slotb = np.zeros((n_win, EDGE_CAP), np.float32)
        colb[wb, pos] = col_loc[sel]
        valb[wb, pos] = vals[sel]
        slotb[wb, pos] = slot_e[sel]
        per_bucket.append((colb, valb, slotb))

    # slot-order output: row r lives at chunk-slot p*SC_H + j where
    # w = window_of_row[r] = c*CW + 3*j + a, p = 32*a + slot_of_row[r]
    c_of = window_of_row // CW
    wl = window_of_row - c_of * CW
    a = wl % GP
    j = wl // GP
    p = 32 * a + slot_of_row
    slot_global = c_of * SLOTS + p * SC_H + j  # [r_per_core]
    return per_bucket, slot_global, n_win


def prep_inputs(adj_rows, adj_cols, adj_vals):
    """Shard + pack. Returns (per-core in_map list, per-core slot maps,
    n_chunks)."""
    adj_rows = np.asarray(adj_rows).astype(np.int64)
    adj_cols = np.asarray(adj_cols).astype(np.int64)
    adj_vals = np.asarray(adj_vals).astype(np.float32)

    bounds = np.searchsorted(adj_rows, np.arange(N_CORES + 1) * R_PER_CORE)
    packed = []
    for k in range(N_CORES):
        e0, e1 = bounds[k], bounds[k + 1]
        rows_l = adj_rows[e0:e1] - k * R_PER_CORE
        packed.append(_pack_core(rows_l, adj_cols[e0:e1],
                                 adj_vals[e0:e1], R_PER_CORE))

    nw_max = max(p[2] for p in packed)
    nw_pad = -(-nw_max // CW) * CW
    n_chunks = nw_pad // CW

    import ml_dtypes
    bf16 = ml_dtypes.bfloat16
    iota = np.broadcast_to(np.arange(SEG_CAP, dtype=np.float32),
                           (128, SEG_CAP)).astype(bf16)
    in_maps = []
    slot_maps = []
    for k in range(N_CORES):
        per_bucket, slot_global, n_win = packed[k]
        m = {"iota": iota}
        for b in range(NB):
            colb, valb, slotb = per_bucket[b]
            cb = np.zeros((nw_pad, EDGE_CAP), np.int16)
            vb = np.zeros((nw_pad, EDGE_CAP), np.float32)
            sb = np.zeros((nw_pad, EDGE_CAP), np.float32)
            cb[:n_win] = colb
            vb[:n_win] = valb
            sb[:n_win] = slotb
            # SBUF-resident layouts (one DMA each):
            # gidx: [128, n_chunks*_GI_W] int16 (16-wrap per chunk, x8)
            m[f"gidx{b}"] = np.concatenate([
                _wrap16(cb[c * CW:(c + 1) * CW].reshape(-1))
                for c in range(n_chunks)], axis=1)
            # vals/slot: [128, n_chunks*CW]; [p, c*CW+t] = edge t*128+p
            m[f"gval{b}"] = np.ascontiguousarray(
                vb.reshape(n_chunks, CW, EDGE_CAP).transpose(2, 0, 1)
                .reshape(128, n_chunks * CW)).astype(bf16)
            m[f"gslot{b}"] = np.ascontiguousarray(
                sb.reshape(n_chunks, CW, EDGE_CAP).transpose(2, 0, 1)
                .reshape(128, n_chunks * CW)).astype(bf16)
        in_maps.append(m)
        slot_maps.append(slot_global)
    return in_maps, slot_maps, n_chunks


def pad_x(x):
    import ml_dtypes
    x2 = np.zeros((N_NODES, XE), ml_dtypes.bfloat16)
    x2[:, :D] = x.astype(ml_dtypes.bfloat16)
    return x2


# ===========================================================================
# Device program (shared across all 8 cores)
# ===========================================================================
def build_program(n_chunks):
    nc = bacc.Bacc("TRN2", target_bir_lowering=False, debug=False,
                   num_devices=N_CORES, num_swdge_queues=4)
    x_d = nc.dram_tensor("x64", [N_NODES, XE], _BF16, kind="ExternalInput")
    gidx_d = [nc.dram_tensor(f"gidx{b}", [128, n_chunks * _GI_W], _I16,
                             kind="ExternalInput") for b in range(NB)]
    gval_d = [nc.dram_tensor(f"gval{b}", [128, n_chunks * CW], _BF16,
                             kind="ExternalInput") for b in range(NB)]
    gslot_d = [nc.dram_tensor(f"gslot{b}", [128, n_chunks * CW], _BF16,
                              kind="ExternalInput") for b in range(NB)]
    iota_d = nc.dram_tensor("iota", [128, SEG_CAP], _BF16,
                            kind="ExternalInput")
    out_d = nc.dram_tensor("out", [n_chunks * SLOTS, D], _F32,
                           kind="ExternalOutput")

    with tile.TileContext(nc) as tc:
        with (
            tc.tile_pool(name="meta", bufs=1) as meta,
            tc.tile_pool(name="gbuf", bufs=4) as gbuf,
            tc.tile_pool(name="sbuf_s", bufs=2) as sbuf_s,
            tc.tile_pool(name="obuf", bufs=2) as obuf,
            tc.tile_pool(name="psum", bufs=4, space="PSUM") as psum,
        ):
            iota_t = meta.tile([128, SEG_CAP], _BF16)
            nc.sync.dma_start(out=iota_t[:], in_=iota_d[:])
            gi_all, gv_all, gs_all = [], [], []
            for b in range(NB):
                gv = meta.tile([128, n_chunks * CW], _BF16, tag=f"gvA{b}")
                gs = meta.tile([128, n_chunks * CW], _BF16, tag=f"gsA{b}")
                nc.sync.dma_start(out=gv[:], in_=gval_d[b][:])
                nc.sync.dma_start(out=gs[:], in_=gslot_d[b][:])
                gv_all.append(gv)
                gs_all.append(gs)
            # chunk-major gidx slices: chunk c's gathers only wait on their
            # own [128, 240] slice, so the pipeline starts ~15us earlier
            for c in range(n_chunks):
                for b in range(NB):
                    gic = meta.tile([128, _GI_W], _I16, tag=f"gi{b}_{c}")
                    nc.sync.dma_start(
                        out=gic[:],
                        in_=gidx_d[b][:, c * _GI_W:(c + 1) * _GI_W])
                    gi_all.append(gic)

            for c in range(n_chunks):
                g_ts, s_ts = [], []
                for b in range(NB):
                    g_t = gbuf.tile([128, CW * XE], _BF16, tag=f"g{b}")
                    nc.gpsimd.dma_gather(
                        out_ap=g_t[:].rearrange("p (t f) -> p t f", f=XE),
                        in_ap=x_d[B_NODES * b:B_NODES * (b + 1)],
                        idxs_ap=gi_all[c * NB + b][:],
                        num_idxs=_NIG, num_idxs_reg=_NIG, elem_size=XE,
                        single_packet=False, queue_num=b,
                    )
                    g_ts.append(g_t)

                    s_t = sbuf_s.tile([128, CW * SEG_CAP], _BF16, tag=f"s{b}")
                    s3 = s_t[:].rearrange("p (t s) -> p t s", s=SEG_CAP)
                    gs_b = gs_all[b][:, c * CW:(c + 1) * CW].unsqueeze(
                        2).to_broadcast([128, CW, SEG_CAP])
                    io_b = iota_t[:].unsqueeze(1).to_broadcast(
                        [128, CW, SEG_CAP])
                    gv_b = gv_all[b][:, c * CW:(c + 1) * CW].unsqueeze(
                        2).to_broadcast([128, CW, SEG_CAP])
                    nc.vector.tensor_tensor(out=s3, in0=gs_b, in1=io_b,
                                            op=mybir.AluOpType.is_equal)
                    nc.vector.tensor_tensor(out=s3, in0=s3, in1=gv_b,
                                            op=mybir.AluOpType.mult)
                    s_ts.append(s_t)

                ps = psum.tile([128, SC_H * D], _F32, space="PSUM", tag="ps")
                for wl in range(CW):
                    a, j = wl % GP, wl // GP
                    for b in range(NB):
                        nc.tensor.matmul(
                            out=ps[32 * a:32 * a + SEG_CAP, D * j:D * j + D],
                            lhsT=s_ts[b][:, SEG_CAP * wl:SEG_CAP * (wl + 1)],
                            rhs=g_ts[b][:, XE * wl:XE * wl + D],
                            start=(b == 0), stop=(b == NB - 1),
                            skip_group_check=True,
                        )

                # PSUM [96, 10, 48] -> SBUF, then one contiguous-ish HWDGE
                # write: chunk-slot p*SC_H+j -> DRAM row c*SLOTS + p*SC_H + j
                o_t = obuf.tile([128, SC_H * D], _F32, tag="o")
                o3 = o_t[:].rearrange("p (j f) -> p j f", f=D)
                ps3 = ps[:].rearrange("p (j f) -> p j f", f=D)
                nc.scalar.copy(out=o3[:96], in_=ps3[:96])
                nc.sync.dma_start(
                    out=out_d[c * SLOTS:(c + 1) * SLOTS].rearrange(
                        "(p j) f -> p j f", j=SC_H),
                    in_=o3[:96],
                )
    nc.compile()
    return nc


# ===========================================================================
# Entry point
# ===========================================================================
_CACHE = {}


def _get_program(n_chunks):
    if n_chunks not in _CACHE:
        _CACHE[n_chunks] = build_program(n_chunks)
    return _CACHE[n_chunks]


def _run(adj_rows, adj_cols, adj_vals, x):
    x64 = pad_x(np.ascontiguousarray(np.asarray(x), dtype=np.float32))
    in_maps, slot_maps, n_chunks = prep_inputs(adj_rows, adj_cols, adj_vals)
    for m in in_maps:
        m["x64"] = x64
    nc = _get_program(n_chunks)
    res = run_bass_kernel_spmd(nc, in_maps, core_ids=list(range(N_CORES)))
    out = np.empty((N_NODES, D), np.float32)
    for k in range(N_CORES):
        out[k * R_PER_CORE:(k + 1) * R_PER_CORE] = \
            res.results[k]["out"][slot_maps[k]]
    return out, res, (in_maps, n_chunks)


def kernel(adj_rows, adj_cols, adj_vals, x):
    out, _, _ = _run(adj_rows, adj_cols, adj_vals, x)
    return out
